# revision 10
# baseline (speedup 1.0000x reference)
"""Trainium2 Bass kernel for nn_Attention_47459388621522.

Computation (B=256, N=2048, D=256):
    hidden = concat([feature, broadcast(pointer_hidden_state)], -1)   # [B,N,2D]
    pre    = tanh(einsum('de,bne->bnd', W[0], hidden))                # [B,N,D]
    scores = einsum('d,bnd->bn', v[0,0], pre)                         # [B,N]
    attns  = softmax(scores, axis=1)[:, None, :]                      # [B,1,N]

Split W = [Wf | Wh] along e: pre = tanh(feature @ Wf^T + bias_b) with
bias = pointer_hidden_state @ Wh^T computed on-device in exact fp32 (tiny).

Sharding: data-parallel over batch, 32 batches per core x 8 cores.

Host prep: feature is transposed per core to a [D, B_PER*N] fp16 "global
token stream" (contraction dim e lands on SBUF partitions; each partition's
DMA row is a long contiguous run). fp16 halves the HBM->SBUF feature
traffic (32 MB/core) vs fp32. The W matmul operand stays float32r (fp32
bits, PE rounds to 12 mantissa bits, 1 cycle/row) -- mixed f32r x fp16
matmuls are supported and keep the W quantization error negligible.

Per-core dataflow (ch_tok tokens per DMA chunk, groups of 512 tokens,
d in 2 chunks of 128):
    PE : pre[d,t]   = sum_ko WfT[e,d]^T @ featT[e,t]   (2 MMs, psum accum)
    ACT: th[d,t]    = tanh(pre + bias[d,b])            (per-partition bias)
    PE : sc[{0,32},t] = v_dc[d,1]^T @ th[d,dc,t]       (2 col-tiled MMs at
         PE column groups 0/32 -> run concurrently; partial scores land on
         psum partitions 0 and 32)
    DVE: psum[0:33,TG] -> stage tile (one copy; free-dim cost only)
    gpsimd DMA: stage row0 -> scoresA[b,:], row32 -> scoresB[b,:]
    softmax over [16, 2048] halves: DVE add A+B, reduce-max, ACT
    exp(x-max)+accum via per-partition bias, DVE reciprocal+scale;
    half 0 overlaps the main loop.
"""

import numpy as np

import concourse.bacc as bacc
import concourse.mybir as mybir
import concourse.tile as tile
from concourse.bass_utils import run_bass_kernel_spmd

f32 = mybir.dt.float32
f32r = mybir.dt.float32r
f16 = mybir.dt.float16

B, N, D = 256, 2048, 256
N_CORES = 8
B_PER = B // N_CORES          # 32 batches per core
TG = 512                      # token group (matmul moving free dim)
NG = N // TG                  # 4 groups per batch
P = 128
DC = D // P                   # 2 d-chunks
KC = D // P                   # 2 e-chunks
TOKS = B_PER * N              # tokens per core

# feature dtype on the wire + as matmul moving operand
FEAT_DT = "f16"               # "f16" or "f32r"
W_DT = "f16"                 # W/v stationary operand dtype ("f32r" only with feat f32r)
SCORE_MODE = "coltile"        # "coltile" (concurrent col-groups) or "seq"
ACT_PAIR = True               # one tanh op per 2 token groups (FD=1024)

_CACHED = {}


def _build(repeat=1, ft_bufs=3, ch_tok=2048, th_bufs=3, scps_bufs=3, stage_bufs=3,
           mmps_bufs=2, mode="full", ft_queues="s", feat_dt=None, w_dt=None,
           score_mode=None, act_pair=ACT_PAIR):
    # ch_tok: tokens per feature DMA chunk (multiple of N).
    # ft_queues: DMA channels for the feature load, round-robin over chunks.
    #            s=SP-HWDGE, a=ACT-HWDGE, p=Pool-SWDGE.
    feat_dt = feat_dt or FEAT_DT
    w_dt = w_dt or W_DT
    score_mode = score_mode or SCORE_MODE
    ft_dt = f16 if feat_dt == "f16" else f32r
    wv_dt = f16 if w_dt == "f16" else f32r
    th_dt = f16 if feat_dt == "f16" else f32r
    assert ch_tok % N == 0 and TOKS % ch_tok == 0
    bat_per_ch = ch_tok // N
    coltile = score_mode == "coltile"
    assert not act_pair or coltile, "act_pair path is coltile-only"
    if act_pair:
        # 3 rotating pre tags x 2 banks = 6 PSUM banks; leave 2 for scores
        scps_bufs = min(scps_bufs, 2)

    nc = bacc.Bacc("TRN2", target_bir_lowering=False, debug=False, name="ptrattn")
    featT = nc.dram_tensor("featT", [D, TOKS], ft_dt, kind="ExternalInput")
    hT = nc.dram_tensor("hT", [D, B_PER], f32, kind="ExternalInput")
    wfT = nc.dram_tensor("wfT", [D, D], wv_dt, kind="ExternalInput")
    whT = nc.dram_tensor("whT", [D, D], f32, kind="ExternalInput")
    vv = nc.dram_tensor("vv", [D, 1], wv_dt, kind="ExternalInput")
    out = nc.dram_tensor("attns", [B_PER, N], f32, kind="ExternalOutput")

    act = mybir.ActivationFunctionType

    with tile.TileContext(nc) as tc:
        with tc.tile_pool(name="singles", bufs=1) as singles, \
             tc.tile_pool(name="feat", bufs=ft_bufs) as feat_pool, \
             tc.tile_pool(name="th", bufs=th_bufs) as th_pool, \
             tc.tile_pool(name="stage", bufs=stage_bufs) as stage_pool, \
             tc.tile_pool(name="soft", bufs=1) as soft_pool, \
             tc.tile_pool(name="mmps", bufs=mmps_bufs, space="PSUM") as mmps, \
             tc.tile_pool(name="scps", bufs=scps_bufs, space="PSUM") as scps:

            # ---- constants (bias inputs first so bias is ready earliest) ----
            wh_full = singles.tile([P, KC, D], f32)
            nc.sync.dma_start(wh_full, whT.rearrange("(ko p) d -> p ko d", p=P))
            hT_sb = singles.tile([P, KC, B_PER], f32)
            nc.sync.dma_start(hT_sb, hT.rearrange("(ko p) b -> p ko b", p=P))
            wf_sb = singles.tile([P, KC, D], wv_dt)
            nc.sync.dma_start(wf_sb, wfT.rearrange("(ko p) d -> p ko d", p=P))
            v_sb = singles.tile([P, DC, 1], wv_dt)
            nc.sync.dma_start(v_sb, vv.rearrange("(ko p) one -> p ko one", p=P))

            # ---- bias[b, d] = Wh @ h_b  (exact fp32, tiny) ----
            # own psum tag so the first main matmuls don't wait on its banks
            bias_sb = singles.tile([P, DC, B_PER], f32)
            for dc in range(DC):
                bias_ps = scps.tile([P, B_PER], f32, tag="sc", bufs=None)
                for ko in range(KC):
                    nc.tensor.matmul(
                        bias_ps,
                        wh_full[:, ko, dc * P:(dc + 1) * P],
                        hT_sb[:, ko, :],
                        start=(ko == 0), stop=(ko == KC - 1),
                    )
                nc.vector.tensor_copy(bias_sb[:, dc, :], bias_ps)

            # scores accumulators, two halves so softmax(half0) overlaps the
            # main loop (DVE ops need base-partition 0, so separate tiles).
            # With coltile scores, each half has an A (psum row 0) and B
            # (psum row 32) tensor summed at softmax time.
            HB = B_PER // 2
            scoresA = [soft_pool.tile([HB, N], f32, name=f"scoresA{h}", tag=f"scoresA{h}")
                       for h in range(2)]
            scoresB = [soft_pool.tile([HB, N], f32, name=f"scoresB{h}", tag=f"scoresB{h}")
                       for h in range(2)] if coltile else None

            def softmax_half(h):
                scores = scoresA[h]
                if coltile:
                    # combine the two col-group partial scores (free-dim cost
                    # only: one [16, 2048] pass)
                    nc.vector.tensor_tensor(
                        scores, scores, scoresB[h], op=mybir.AluOpType.add)
                negmax = soft_pool.tile([HB, 1], f32, tag=f"negmax{h}")
                nc.vector.tensor_reduce(
                    negmax, scores, axis=mybir.AxisListType.X,
                    op=mybir.AluOpType.max, negate=True)
                # exp(score - max) fused via per-partition bias; the ACT exp
                # LUT underflows cleanly to 0 for very negative inputs
                # (probed down to -10000), so no clamp pass is needed
                probs = soft_pool.tile([HB, N], f32, tag=f"probs{h}")
                sumexp = soft_pool.tile([HB, 1], f32, tag=f"sumexp{h}")
                nc.scalar.activation(
                    probs, scores, act.Exp, bias=negmax, scale=1.0,
                    accum_out=sumexp)
                rcp = soft_pool.tile([HB, 1], f32, tag=f"rcp{h}")
                nc.vector.reciprocal(rcp, sumexp)
                nc.vector.tensor_scalar_mul(probs, probs, rcp)
                nc.gpsimd.dma_start(out.ap()[h * HB:(h + 1) * HB, :], probs)

            # ---- main loop over feature chunks ----
            qmap = {"s": nc.sync, "a": nc.scalar, "p": nc.gpsimd}
            featT_r = featT.rearrange("(ko p) t -> p ko t", p=P)
            for rep in range(repeat):
                for ch in range(TOKS // ch_tok):
                    ft = feat_pool.tile([P, KC, ch_tok], ft_dt, tag="ft")
                    eng = qmap[ft_queues[ch % len(ft_queues)]]
                    ft_src = featT_r[:, :, ch * ch_tok:(ch + 1) * ch_tok]
                    if ch == 0 and rep == 0:
                        # split the first load so the pipeline starts on the
                        # first quarter instead of waiting for the full chunk
                        q = ch_tok // 4
                        for s in range(4):
                            eng.dma_start(ft[:, :, s * q:(s + 1) * q],
                                          ft_src[:, :, s * q:(s + 1) * q])
                    else:
                        eng.dma_start(ft, ft_src)

                    for bl in range(bat_per_ch):
                        b = ch * bat_per_ch + bl
                        h, row = divmod(b, HB)
                        if mode == "dma_only":
                            stage = stage_pool.tile([1, N], f32, tag="stage")
                            nc.vector.tensor_copy(stage[:, 0:8], ft[0:1, 0, 0:8].bitcast(f32))
                            nc.gpsimd.dma_start(
                                scoresA[h][row:row + 1, 0:2], stage[:, 0:2])
                            if coltile:
                                nc.gpsimd.dma_start(
                                    scoresB[h][row:row + 1, 0:2], stage[:, 0:2])
                            if row == HB - 1:
                                softmax_half(h)
                            continue
                        stage = stage_pool.tile(
                            [33 if coltile else 1, NG, TG], f32, tag="stage")
                        if act_pair:
                            # two groups per ACT op (FD=2*TG amortizes the
                            # per-instruction ScalarE overhead); pre tiles
                            # rotate over 3 tags (2 banks each) so PE runs
                            # ahead of ACT by 1.5 pairs
                            for pair in range(NG // 2):
                                base = bl * N + pair * 2 * TG
                                th = th_pool.tile([P, DC, 2, TG], th_dt, tag="th")
                                for dc in range(DC):
                                    pidx = (b * NG + pair * 2 + dc) % 3
                                    pre = mmps.tile([P, 2, TG], f32,
                                                    tag=f"pre{pidx}", bufs=1)
                                    for gp in range(2):
                                        ts = slice(base + gp * TG,
                                                   base + (gp + 1) * TG)
                                        for ko in range(KC):
                                            nc.tensor.matmul(
                                                pre[:, gp, :],
                                                wf_sb[:, ko, dc * P:(dc + 1) * P],
                                                ft[:, ko, ts],
                                                start=(ko == 0), stop=(ko == KC - 1),
                                            )
                                    nc.scalar.activation(
                                        th[:, dc, :, :], pre, act.Tanh,
                                        bias=bias_sb[:, dc, b:b + 1], scale=1.0)
                                for gp in range(2):
                                    g = pair * 2 + gp
                                    sc = scps.tile([33, TG], f32, tag="sc")
                                    for dc in range(DC):
                                        nc.tensor.matmul(
                                            sc[32 * dc:32 * dc + 1, :],
                                            v_sb[:, dc, :], th[:, dc, gp, :],
                                            start=True, stop=True,
                                        )
                                    nc.vector.tensor_copy(stage[:, g, :], sc)
                        else:
                            for g in range(NG):
                                ts = slice(bl * N + g * TG, bl * N + (g + 1) * TG)
                                th = th_pool.tile([P, DC, TG], th_dt, tag="th")
                                for dc in range(DC):
                                    pre = mmps.tile([P, TG], f32, tag=f"pre{dc}")
                                    for ko in range(KC):
                                        nc.tensor.matmul(
                                            pre,
                                            wf_sb[:, ko, dc * P:(dc + 1) * P],
                                            ft[:, ko, ts],
                                            start=(ko == 0), stop=(ko == KC - 1),
                                        )
                                    nc.scalar.activation(
                                        th[:, dc, :], pre, act.Tanh,
                                        bias=bias_sb[:, dc, b:b + 1], scale=1.0)
                                if coltile:
                                    # two concurrent MMs on PE col groups 0/32
                                    sc = scps.tile([33, TG], f32, tag="sc")
                                    for dc in range(DC):
                                        nc.tensor.matmul(
                                            sc[32 * dc:32 * dc + 1, :],
                                            v_sb[:, dc, :], th[:, dc, :],
                                            start=True, stop=True,
                                        )
                                    nc.vector.tensor_copy(stage[:, g, :], sc)
                                else:
                                    sc = scps.tile([1, TG], f32, tag="sc")
                                    for dc in range(DC):
                                        nc.tensor.matmul(
                                            sc, v_sb[:, dc, :], th[:, dc, :],
                                            start=(dc == 0), stop=(dc == DC - 1),
                                        )
                                    nc.vector.tensor_copy(stage[0:1, g, :], sc)
                        # separate queue from the ft loads (no head-of-line block)
                        nc.gpsimd.dma_start(
                            scoresA[h][row:row + 1, :],
                            stage[0:1, :, :].rearrange("a g t -> a (g t)"))
                        if coltile:
                            nc.gpsimd.dma_start(
                                scoresB[h][row:row + 1, :],
                                stage[32:33, :, :].rearrange("a g t -> a (g t)"))
                        if row == HB - 1:
                            softmax_half(h)

    nc.compile()
    return nc


def _host_prep(feature, pointer_hidden_state, v, W, feat_dt=None, w_dt=None):
    feat_dt = feat_dt or FEAT_DT
    w_dt = w_dt or W_DT
    ft_np = np.float16 if feat_dt == "f16" else np.float32
    wv_np = np.float16 if w_dt == "f16" else np.float32
    Wf = W[0][:, :D]
    whT = np.ascontiguousarray(W[0][:, D:].T.astype(np.float32))   # [e, d]
    wfT = np.ascontiguousarray(Wf.T.astype(wv_np))                 # [e, d]
    vv = np.ascontiguousarray(v[0, 0][:, None].astype(wv_np))      # [D, 1]
    per_core = []
    for c in range(N_CORES):
        sl = slice(c * B_PER, (c + 1) * B_PER)
        # [D, B_PER*N] global token stream: featT[e, b*N+n] = feature[b, n, e]
        featT = np.ascontiguousarray(
            feature[sl].astype(ft_np).transpose(2, 0, 1).reshape(D, TOKS))
        hT = np.ascontiguousarray(pointer_hidden_state[sl].T.astype(np.float32))
        per_core.append({"featT": featT, "hT": hT, "wfT": wfT, "whT": whT, "vv": vv})
    return per_core


def kernel(feature, pointer_hidden_state, v, W):
    feature = np.asarray(feature)
    pointer_hidden_state = np.asarray(pointer_hidden_state)
    v = np.asarray(v)
    W = np.asarray(W)

    if "nc" not in _CACHED:
        _CACHED["nc"] = _build()
    nc = _CACHED["nc"]

    in_maps = _host_prep(feature, pointer_hidden_state, v, W)
    res = run_bass_kernel_spmd(nc, in_maps, core_ids=list(range(N_CORES)))
    _CACHED["last_res"] = res
    outs = [res.results[c]["attns"] for c in range(N_CORES)]
    return np.concatenate(outs, axis=0)[:, None, :].astype(np.float32)
